# revision 20
# baseline (speedup 1.0000x reference)
"""Decoder attention (QKV proj + KV-cache scatter + full softmax attention + out proj)
on 8 Trainium2 cores.

Sharding: core = (batch b, head-group g).  b = core//2, g = core%2; each core
handles 8 of the 16 heads for one batch element.  The reference's
masked_scatter is equivalent to attending over concat([k_new, cache_keep])
(softmax is permutation-invariant over keys); cache_keep rows are gathered
host-side, so no on-device scatter is needed.

Device schedule (ACT-bound pipeline):
  The softmax exp is ScalarE-only at 1 elem/cycle/lane -> 8 heads x 16 kv
  j-tiles x [128,1024] exps ~= 132 us is the hard floor.  Everything else
  is scheduled to hide inside it:

  - heads processed in PAIRS (2hp, 2hp+1) living at partitions 0-63 /
    64-127 of shared q/k tiles.  Per (pair, j): scores A (2 matmuls, K=64,
    PE row-groups 0-1) -> exp A; scores B (row-groups 2-3) -> exp B.  ACT
    alternates A/B exps with no idle; PE fills its slack with lagged AV,
    denominator matmuls, and drained QKV/proj thunks.
  - AV lags by `av_lag` steps: per (j,cch) the two heads' AV matmuls
    (K=128, M=64) write disjoint partition halves (PE col-groups 0-1 vs
    2-3) of ONE [128,1024] PSUM tile, so they can run concurrently.
  - softmax denominators via ones-vector matmuls (M=1) col-packed 4-way at
    out partitions {0,32,64,96} of a 1-bank accumulator.  av/den banks are
    pre-initialized by a K=1 matmul (av: zeros, den: ones) so has_written
    covers each bank and the chains (start=False) can share banks safely;
    den's extra +1 is subtracted exactly during evacuation.

  PSUM: spsA(2) + spsB(2) + av(2) + den(1) + qps(1) = 8 banks exactly.
"""

import os
import sys

for _p in ("/opt/trn_rl_repo", "/root/.axon_site/_ro/trn_rl_repo"):
    if os.path.isdir(_p) and _p not in sys.path:
        sys.path.insert(0, _p)
        break

import numpy as np

import concourse.bacc as bacc
import concourse.mybir as mybir
import concourse.tile as tile
from concourse import bass_utils

B, NX, NC, C, H = 4, 1024, 2048, 1024, 16
DH = C // H                      # 64
G = 2                            # head groups (tensor-parallel factor)
HPG = H // G                     # 8 heads per group
NP = HPG // 2                    # 4 head pairs per core
CG = HPG * DH                    # 512 channels per group
SCALE = DH ** -0.5
N_CORES = 8
NJ = NC // 128                   # 16 kv j-tiles
F32 = mybir.dt.float32
BF16 = mybir.dt.bfloat16
EXP = mybir.ActivationFunctionType.Exp

_STATE = {}


def _build(reps: int = 1):
    nc = bacc.Bacc("TRN2", target_bir_lowering=False, debug=False)

    xT_d = nc.dram_tensor("xT", [C, NX], BF16, kind="ExternalInput")
    wqkT_d = nc.dram_tensor("wqkT", [C, 2 * CG], BF16, kind="ExternalInput")
    wvT_d = nc.dram_tensor("wvT", [C, CG], BF16, kind="ExternalInput")
    bqk_d = nc.dram_tensor("bqk", [128, 8], F32, kind="ExternalInput")
    bv_d = nc.dram_tensor("bv", [128, CG], F32, kind="ExternalInput")
    kkeepT_d = nc.dram_tensor("kkeepT", [CG, NC - NX], BF16, kind="ExternalInput")
    vkeep_d = nc.dram_tensor("vkeep", [NC - NX, CG], BF16, kind="ExternalInput")
    wprojT_d = nc.dram_tensor("wprojT", [CG, C], BF16, kind="ExternalInput")
    outT_d = nc.dram_tensor("outT", [C, NX], F32, kind="ExternalOutput")

    with tile.TileContext(nc) as tc:
        with (
            tc.tile_pool(name="persist", bufs=1) as pp,
            tc.tile_pool(name="work", bufs=1) as wp,
            tc.tile_pool(name="wqkc", bufs=8) as wqkp,
            tc.tile_pool(name="attn", bufs=8) as ep,
            tc.tile_pool(name="nrm", bufs=2) as np_pool,
            tc.tile_pool(name="out_sb", bufs=1) as op,
            tc.tile_pool(name="ps", bufs=1, space="PSUM") as psp,
        ):
            # ---- persistent SBUF tiles ----
            q_t = [pp.tile([128, NX], BF16, tag=f"q{i}", name=f"q{i}") for i in range(NP)]
            kbig = pp.tile([128, NP * NC], BF16, tag="kbig")
            k_t = [kbig[:, i * NC:(i + 1) * NC] for i in range(NP)]
            vbig = pp.tile([128, NJ * CG], BF16, tag="vbig")
            v_t = [vbig[:, j * CG:(j + 1) * CG] for j in range(NJ)]
            # partition-swapped copies of q/k (head pair halves exchanged):
            # let each head's scores matmuls alternate PE row groups 0-1/2-3,
            # which runs ~2x faster than same-group back-to-back issue.
            kdbig = pp.tile([128, NP * NC], BF16, tag="kdbig")
            kd_t = [kdbig[:, i * NC:(i + 1) * NC] for i in range(NP)]
            qd_t = [pp.tile([128, NX], BF16, tag=f"qd{i}", name=f"qd{i}")
                    for i in range(NP)]
            a_t = [pp.tile([128, NX], BF16, tag=f"a{i}", name=f"a{i}") for i in range(NP)]
            bqk_t = pp.tile([128, 8], F32, tag="bqk")
            bv_t = pp.tile([128, CG], F32, tag="bv")
            xT_t = [wp.tile([128, NX], BF16, tag=f"x{i}", name=f"x{i}") for i in range(8)]
            wvbig = wp.tile([128, 8 * CG], BF16, tag="wvbig")
            wv_t = [wvbig[:, i * CG:(i + 1) * CG] for i in range(8)]
            ones_t = pp.tile([128, 1], BF16, tag="ones")      # denominator lhsT
            onesq_t = pp.tile([128, 128], BF16, tag="onesq")  # init lhsT (all 1)
            erow_t = pp.tile([128, 512], BF16, tag="erow")    # e0 rhs (row0=1)
            zr_t = pp.tile([128, 512], BF16, tag="zr")        # zeros rhs
            proj_sb = [pp.tile([128, NX], F32, tag=f"pj{m}", name=f"pj{m}")
                       for m in range(8)]

            nc.sync.dma_start(bqk_t[:], bqk_d.ap())
            nc.sync.dma_start(bv_t[:], bv_d.ap())

            def body():
                # constants + ACT exp-table preload (hidden under input DMA)
                nc.vector.memset(ones_t[:], 1.0)
                nc.vector.memset(onesq_t[:], 1.0)
                nc.vector.memset(zr_t[:], 0.0)
                nc.vector.memset(erow_t[:], 0.0)
                nc.vector.memset(erow_t[0:1, :], 1.0)
                scr0 = wp.tile([128, 8], F32, tag="scr0")
                scr1 = wp.tile([128, 8], F32, tag="scr1")
                nc.vector.memset(scr0[:], 0.0)
                nc.scalar.activation(scr1[:], scr0[:], EXP)

                # ---- input DMAs, spread across engine DGE queues so the
                # transfers overlap: SP carries the prologue-critical data
                # (xT low half, kkeep0, then the wqk m-tile batches inside the
                # prologue thunks); Pool/ACT/DVE carry the rest in batched
                # strided transfers.
                for i in range(4):
                    nc.sync.dma_start(xT_t[i][:], xT_d[i * 128:(i + 1) * 128, :])
                nc.sync.dma_start(k_t[0][0:128, NX:NC], kkeepT_d[0:128, :])
                for i in range(4, 8):
                    nc.gpsimd.dma_start(xT_t[i][:], xT_d[i * 128:(i + 1) * 128, :])
                nc.gpsimd.dma_start(
                    wvbig[:].rearrange("p (kk c) -> p kk c", kk=8),
                    wvT_d[0:C, :].rearrange("(kk p) c -> p kk c", p=128),
                )
                nc.scalar.dma_start(
                    vbig[:, (NJ // 2) * CG:].rearrange("p (jj c) -> p jj c", jj=8),
                    vkeep_d[0:NC - NX, :].rearrange("(jj p) c -> p jj c", p=128),
                )
                nc.scalar.dma_start(
                    kbig[:].rearrange("p (i n) -> p i n", i=NP)[:, 1:NP, NX:NC],
                    kkeepT_d[128:CG, :].rearrange("(i p) n -> p i n", p=128),
                )

                # ---------- thunk generators ----------
                def qk_thunks(m_list):
                    """QKV q/k m-tiles: m<4 -> q_t[m], m>=4 -> k_t[m-4][:,0:NX].
                    One [128,512] PSUM chunk (bank `qps`) per cch; the m-tile's
                    8 weight K-chunks arrive in ONE strided DMA."""
                    for m in m_list:
                        wqk_m = wqkp.tile([128, 1024], BF16, tag="wqkc",
                                          bufs=3, name=f"wqkm{m}")

                        def wdma(m=m, wqk_m=wqk_m):
                            nc.sync.dma_start(
                                wqk_m[:].rearrange("p (kk c) -> p kk c", kk=8),
                                wqkT_d[0:C, m * 128:(m + 1) * 128]
                                .rearrange("(kk p) c -> p kk c", p=128),
                            )
                        yield wdma
                        for cch in range(2):
                            qps = psp.tile([128, 512], F32, tag="qps", bufs=1,
                                           name=f"qps{m}_{cch}")
                            for kk in range(8):
                                def mm(cch=cch, kk=kk, qps=qps, wqk_m=wqk_m):
                                    nc.tensor.matmul(
                                        qps[:],
                                        wqk_m[:, kk * 128:(kk + 1) * 128],
                                        xT_t[kk][:, cch * 512:(cch + 1) * 512],
                                        start=(kk == 0),
                                        stop=(kk == 7),
                                    )
                                yield mm
                            def bias(m=m, cch=cch, qps=qps):
                                sl = slice(cch * 512, (cch + 1) * 512)
                                dest = q_t[m][:, sl] if m < 4 else k_t[m - 4][:, sl]
                                nc.vector.tensor_scalar_add(dest, qps[:],
                                                            bqk_t[:, m:m + 1])
                            yield bias

                def v_thunks():
                    """V projection m-tiles -> v_t[m] (token-major, channel
                    cols h*64+d)."""
                    for m in range(8):
                        vps = psp.tile([128, 512], F32, tag="qps", bufs=1,
                                       name=f"vps{m}")
                        for kk in range(8):
                            def mm(m=m, kk=kk, vps=vps):
                                nc.tensor.matmul(
                                    vps[:],
                                    xT_t[kk][:, m * 128:(m + 1) * 128],
                                    wv_t[kk][:],
                                    start=(kk == 0),
                                    stop=(kk == 7),
                                )
                            yield mm
                        def fin(m=m, vps=vps):
                            nc.vector.tensor_add(v_t[m][:], vps[:], bv_t[:])
                        yield fin

                def proj_thunks(pairs, first):
                    """Partial output projection over pair K-chunks; partials
                    accumulated in proj_sb (f32 SBUF) via DVE."""
                    for m in range(8):
                        chunks = []
                        for cch in range(2):
                            pps = psp.tile([128, 512], F32, tag="qps", bufs=1,
                                           name=f"pps{m}_{cch}")
                            for i, p in enumerate(pairs):
                                def mm(m=m, cch=cch, p=p, i=i, pps=pps,
                                       chunks=chunks, n=len(pairs)):
                                    if cch == 0:
                                        wpc = op.tile([128, 128], BF16,
                                                      tag="wpc", bufs=8,
                                                      name=f"wpc{p}_{m}")
                                        nc.sync.dma_start(
                                            wpc[:],
                                            wprojT_d[p * 128:(p + 1) * 128,
                                                     m * 128:(m + 1) * 128],
                                        )
                                        chunks.append(wpc)
                                    else:
                                        wpc = chunks[i]
                                    nc.tensor.matmul(
                                        pps[:],
                                        wpc[:],
                                        a_t[p][:, cch * 512:(cch + 1) * 512],
                                        start=(i == 0),
                                        stop=(i == n - 1),
                                    )
                                yield mm
                            def fin(m=m, cch=cch, pps=pps, first=first):
                                sl = slice(cch * 512, (cch + 1) * 512)
                                if first:
                                    nc.vector.tensor_copy(proj_sb[m][:, sl],
                                                          pps[:])
                                else:
                                    nc.vector.tensor_add(proj_sb[m][:, sl],
                                                         proj_sb[m][:, sl],
                                                         pps[:])
                            yield fin

                # Drained thunks provide NO automatic ordering: Tile deps
                # follow program order, so a consumer emitted before its
                # producer thunk reads stale/uninitialized data.  Thunks are
                # tagged with a completion marker; need(marker) force-drains
                # (in FIFO order, keeping PSUM chains contiguous) until that
                # marker's thunks have all been emitted.
                pending = []          # list of (marker_or_None, thunk)
                done = set()

                def _pop():
                    marker, th = pending.pop(0)
                    th()
                    if marker:
                        done.add(marker)

                def drain(n):
                    for _ in range(n):
                        if pending:
                            _pop()

                def need(marker):
                    while marker not in done and pending:
                        _pop()

                def tag_last(marker, thunks):
                    thunks = list(thunks)
                    return [(None, th) for th in thunks[:-1]] + \
                           [(marker, thunks[-1])]

                # ---------- prologue: q/k for pair 0 ----------
                for th in qk_thunks([0, 4]):
                    th()

                pending += tag_last("qk1", qk_thunks([1, 5]))
                vt = list(v_thunks())
                for m in range(8):
                    pending += tag_last(f"v{m}", vt[m * 9:(m + 1) * 9])
                pending += tag_last("qk2", qk_thunks([2, 6]))
                pending += tag_last("qk3", qk_thunks([3, 7]))

                # ---------- attention over head pairs ----------
                for hp in range(NP):
                    if hp >= 1:
                        need(f"qk{hp}")        # q_t/k_t[hp] producers
                    av_lag = 4 if hp == 0 else 1
                    d1, d2 = (3, 3) if hp == 0 else (1, 2)
                    jo = (list(range(NJ // 2, NJ)) + list(range(NJ // 2))
                          if hp == 0 else list(range(NJ)))
                    av = psp.tile([128, NX], F32, tag="av", bufs=1,
                                  name=f"av{hp}")
                    den = psp.tile([128, 512], F32, tag="den", bufs=1,
                                   name=f"den{hp}")
                    # bank init matmuls: av <- ones.T@0 = 0, den <- ones.T@e0 = 1
                    for cch in range(2):
                        nc.tensor.matmul(
                            av[:, cch * 512:(cch + 1) * 512],
                            onesq_t[:], zr_t[:], start=True, stop=True,
                        )
                    nc.tensor.matmul(den[:], onesq_t[:], erow_t[:],
                                     start=True, stop=True)

                    ets = {}

                    def av_den(j, last, hp=hp, av=av, den=den, ets=ets):
                        """AV pair (col-groups 0-1 vs 2-3, concurrent) +
                        4-way col-packed denominator matmuls."""
                        if j < NJ // 2:
                            need(f"v{j}")      # v_t[j] producer must precede
                        etA, etB = ets.pop(j)
                        for cch in range(2):
                            sl = slice(cch * 512, (cch + 1) * 512)
                            for half, et in ((0, etA), (1, etB)):
                                h = 2 * hp + half
                                nc.tensor.matmul(
                                    av[half * 64:(half + 1) * 64, sl],
                                    v_t[j][:, h * 64:(h + 1) * 64],
                                    et[:, sl],
                                    start=False, stop=(last and cch == 1),
                                    skip_group_check=True,
                                )
                        for idx, (et, cch) in enumerate(
                                ((etA, 0), (etA, 1), (etB, 0), (etB, 1))):
                            nc.tensor.matmul(
                                den[idx * 32:idx * 32 + 1, :],
                                ones_t[:],
                                et[:, cch * 512:(cch + 1) * 512],
                                start=False, stop=last,
                                skip_group_check=True,
                                tile_position=(0, idx * 32),
                            )

                    for step, j in enumerate(jo):
                        spsA = psp.tile([128, NX], F32, tag="spsA", bufs=1,
                                        name=f"spsA{hp}_{j}")
                        spsB = psp.tile([128, NX], F32, tag="spsB", bufs=1,
                                        name=f"spsB{hp}_{j}")
                        etA = ep.tile([128, NX], BF16, tag="etA",
                                      name=f"etA{hp}_{j}")
                        etB = ep.tile([128, NX], BF16, tag="etB",
                                      name=f"etB{hp}_{j}")
                        ets[j] = (etA, etB)
                        for cch in range(2):
                            nc.tensor.matmul(
                                spsA[:, cch * 512:(cch + 1) * 512],
                                k_t[hp][0:64, j * 128:(j + 1) * 128],
                                q_t[hp][0:64, cch * 512:(cch + 1) * 512],
                                start=True, stop=True,
                            )
                        nc.scalar.activation(etA[:], spsA[:], EXP, scale=SCALE)
                        if step >= av_lag:
                            av_den(jo[step - av_lag], last=False)
                        drain(d1)
                        for cch in range(2):
                            nc.tensor.matmul(
                                spsB[:, cch * 512:(cch + 1) * 512],
                                k_t[hp][64:128, j * 128:(j + 1) * 128],
                                q_t[hp][64:128, cch * 512:(cch + 1) * 512],
                                start=True, stop=True,
                            )
                        nc.scalar.activation(etB[:], spsB[:], EXP, scale=SCALE)
                        drain(d2)
                    for step in range(NJ - av_lag, NJ):
                        av_den(jo[step], last=(step == NJ - 1))
                        drain(1)

                    # ---- normalization for this pair ----
                    avs = np_pool.tile([128, NX], F32, tag="avs", bufs=2,
                                       name=f"avs{hp}")
                    nc.vector.tensor_copy(avs[:], av[:])
                    dens = np_pool.tile([128, 512], F32, tag="dens", bufs=2,
                                        name=f"dens{hp}")
                    # dens = max(den - 1, tiny): removes the bank-init +1
                    # exactly; `tiny` keeps never-written rows finite under
                    # reciprocal.
                    nc.vector.tensor_scalar(
                        dens[:], den[:], 1.0, 1e-30,
                        op0=mybir.AluOpType.subtract,
                        op1=mybir.AluOpType.max,
                    )
                    # gpsimd partition_broadcast only honors sources at
                    # partition 0 on HW, so first move each denominator row
                    # into segments of a partition-0 tile via 1-partition DVE
                    # reciprocals (DVE handles mismatched base partitions when
                    # the partition counts match).
                    d4 = np_pool.tile([1, 4 * 512], F32, tag="d4", bufs=2,
                                      name=f"d4{hp}")
                    for r in range(4):
                        nc.vector.reciprocal(
                            d4[0:1, r * 512:(r + 1) * 512],
                            dens[r * 32:r * 32 + 1, :],
                        )
                    rb = np_pool.tile([128, NX], F32, tag="rb", bufs=2,
                                      name=f"rb{hp}")
                    for half in range(2):
                        for cch in range(2):
                            r = half * 2 + cch
                            nc.gpsimd.partition_broadcast(
                                rb[half * 64:(half + 1) * 64,
                                   cch * 512:(cch + 1) * 512],
                                d4[0:1, r * 512:(r + 1) * 512],
                            )
                    nc.vector.tensor_mul(a_t[hp][:], avs[:], rb[:])

                    if hp == 2:
                        pending += [(None, th)
                                    for th in proj_thunks([0, 1, 2], first=True)]

                while pending:
                    _pop()

                # ---------- tail: last proj K-chunk + store ----------
                for m in range(8):
                    wpc = op.tile([128, 128], BF16, tag="wpc", bufs=8,
                                  name=f"wpcT{m}")
                    nc.sync.dma_start(
                        wpc[:],
                        wprojT_d[3 * 128:4 * 128, m * 128:(m + 1) * 128],
                    )
                    for cch in range(2):
                        pps = psp.tile([128, 512], F32, tag="qps", bufs=1,
                                       name=f"ppsT{m}_{cch}")
                        nc.tensor.matmul(
                            pps[:], wpc[:],
                            a_t[3][:, cch * 512:(cch + 1) * 512],
                            start=True, stop=True,
                        )
                        ot = op.tile([128, 512], F32, tag="ot", bufs=4)
                        nc.vector.tensor_add(
                            ot[:], pps[:],
                            proj_sb[m][:, cch * 512:(cch + 1) * 512])
                        nc.sync.dma_start(
                            outT_d[m * 128:(m + 1) * 128,
                                   cch * 512:(cch + 1) * 512],
                            ot[:],
                        )

            if reps == 1:
                body()
            else:
                hints = (
                    mybir.EngineType.PE,
                    mybir.EngineType.Activation,
                    mybir.EngineType.DVE,
                    mybir.EngineType.SP,
                )
                with tc.For_i(0, reps, 1, hint_engines=hints):
                    body()

    nc.compile()
    return nc


def _get_nc():
    if "nc" not in _STATE:
        _STATE["nc"] = _build()
    return _STATE["nc"]


def _prep_in_maps(x, update_idx, cache_k, cache_v, w_qkv, b_qkv):
    """Host-side sharding: build the 8 per-core input dicts."""
    import ml_dtypes

    x = np.asarray(x, np.float32)
    update_idx = np.asarray(update_idx)
    cache_k = np.asarray(cache_k, np.float32)
    cache_v = np.asarray(cache_v, np.float32)
    w_qkv = np.asarray(w_qkv, np.float32)
    b_qkv = np.asarray(b_qkv, np.float32)

    per_g = []
    for g in range(G):
        qs = slice(g * CG, (g + 1) * CG)
        ks = slice(C + g * CG, C + (g + 1) * CG)
        vs = slice(2 * C + g * CG, 2 * C + (g + 1) * CG)
        wqkT = np.ascontiguousarray(
            np.concatenate([w_qkv[qs], w_qkv[ks]], 0).T
        ).astype(ml_dtypes.bfloat16)                                 # (C, 2CG)
        wvT = np.ascontiguousarray(w_qkv[vs].T).astype(ml_dtypes.bfloat16)
        bqk = np.ascontiguousarray(
            np.concatenate([b_qkv[qs], b_qkv[ks]]).reshape(8, 128).T
        )                                                            # (128, 8)
        bv = np.broadcast_to(b_qkv[vs][None, :], (128, CG)).copy()
        per_g.append((wqkT, wvT, bqk, bv))

    in_maps = []
    for b in range(B):
        idx = update_idx[b]
        mask = np.ones(NC, bool)
        mask[idx] = False
        keep = np.nonzero(mask)[0]                                   # sorted
        xT = np.ascontiguousarray(x[b].T).astype(ml_dtypes.bfloat16)  # (C, NX)
        for g in range(G):
            wqkT, wvT, bqk, bv = per_g[g]
            hsel = slice(g * HPG, (g + 1) * HPG)
            kk = cache_k[b, hsel][:, keep, :]                        # (HPG, NC-NX, DH)
            kkeepT = np.ascontiguousarray(
                kk.transpose(0, 2, 1).reshape(HPG * DH, NC - NX)
            ).astype(ml_dtypes.bfloat16)
            vkeep = np.ascontiguousarray(
                cache_v[b, hsel][:, keep, :].transpose(1, 0, 2)
                .reshape(NC - NX, CG)
            ).astype(ml_dtypes.bfloat16)
            wprojT = np.asarray(_STATE["wprojT"][g], ml_dtypes.bfloat16)
            in_maps.append(
                dict(
                    xT=xT, wqkT=wqkT, wvT=wvT, bqk=bqk, bv=bv,
                    kkeepT=kkeepT, vkeep=vkeep, wprojT=wprojT,
                )
            )
    return in_maps


def kernel(x, update_idx, cache_k, cache_v, w_qkv, b_qkv, w_proj, b_proj):
    nc = _get_nc()
    w_proj = np.asarray(w_proj, np.float32)
    b_proj = np.asarray(b_proj, np.float32)
    _STATE["wprojT"] = [
        np.ascontiguousarray(w_proj[:, g * CG:(g + 1) * CG].T) for g in range(G)
    ]
    in_maps = _prep_in_maps(x, update_idx, cache_k, cache_v, w_qkv, b_qkv)
    res = bass_utils.run_bass_kernel_spmd(nc, in_maps, core_ids=list(range(N_CORES)))
    _STATE["last_results"] = res
    out = np.empty((B, NX, C), np.float32)
    for b in range(B):
        acc = res.results[2 * b]["outT"] + res.results[2 * b + 1]["outT"]
        out[b] = acc.T + b_proj
    return out


# revision 24
# speedup vs baseline: 1.0005x; 1.0005x over previous
"""Decoder attention (QKV proj + KV-cache scatter + full softmax attention + out proj)
on 8 Trainium2 cores.

Sharding: core = (batch b, head-group g).  b = core//2, g = core%2; each core
handles 8 of the 16 heads for one batch element.  The reference's
masked_scatter is equivalent to attending over concat([k_new, cache_keep])
(softmax is permutation-invariant over keys); cache_keep rows are gathered
host-side, so no on-device scatter is needed.

Device schedule (ACT-bound pipeline):
  The softmax exp is ScalarE-only at 1 elem/cycle/lane -> 8 heads x 16 kv
  j-tiles x [128,1024] exps ~= 132 us is the hard floor.  Everything else
  is scheduled to hide inside it:

  - heads processed in PAIRS (2hp, 2hp+1) living at partitions 0-63 /
    64-127 of shared q/k tiles.  Per (pair, j): scores A (2 matmuls, K=64,
    PE row-groups 0-1) -> exp A; scores B (row-groups 2-3) -> exp B.  ACT
    alternates A/B exps with no idle; PE fills its slack with lagged AV,
    denominator matmuls, and drained QKV/proj thunks.
  - AV lags by `av_lag` steps: per (j,cch) the two heads' AV matmuls
    (K=128, M=64) write disjoint partition halves (PE col-groups 0-1 vs
    2-3) of ONE [128,1024] PSUM tile, so they can run concurrently.
  - softmax denominators via ones-vector matmuls (M=1) col-packed 4-way at
    out partitions {0,32,64,96} of a 1-bank accumulator.  av/den banks are
    pre-initialized by a K=1 matmul (av: zeros, den: ones) so has_written
    covers each bank and the chains (start=False) can share banks safely;
    den's extra +1 is subtracted exactly during evacuation.

  PSUM: spsA(2) + spsB(2) + av(2) + den(1) + qps(1) = 8 banks exactly.
"""

import os
import sys

for _p in ("/opt/trn_rl_repo", "/root/.axon_site/_ro/trn_rl_repo"):
    if os.path.isdir(_p) and _p not in sys.path:
        sys.path.insert(0, _p)
        break

import numpy as np

import concourse.bacc as bacc
import concourse.mybir as mybir
import concourse.tile as tile
from concourse import bass_utils

B, NX, NC, C, H = 4, 1024, 2048, 1024, 16
DH = C // H                      # 64
G = 2                            # head groups (tensor-parallel factor)
HPG = H // G                     # 8 heads per group
NP = HPG // 2                    # 4 head pairs per core
CG = HPG * DH                    # 512 channels per group
SCALE = DH ** -0.5
N_CORES = 8
NJ = NC // 128                   # 16 kv j-tiles
F32 = mybir.dt.float32
BF16 = mybir.dt.bfloat16
EXP = mybir.ActivationFunctionType.Exp

_STATE = {}


def _build(reps: int = 1):
    nc = bacc.Bacc("TRN2", target_bir_lowering=False, debug=False)

    xT_d = nc.dram_tensor("xT", [C, NX], BF16, kind="ExternalInput")
    wqkT_d = nc.dram_tensor("wqkT", [C, 2 * CG], BF16, kind="ExternalInput")
    wvT_d = nc.dram_tensor("wvT", [C, CG], BF16, kind="ExternalInput")
    bqk_d = nc.dram_tensor("bqk", [128, 8], F32, kind="ExternalInput")
    bv_d = nc.dram_tensor("bv", [128, CG], F32, kind="ExternalInput")
    kkeepT_d = nc.dram_tensor("kkeepT", [CG, NC - NX], BF16, kind="ExternalInput")
    vkeep_d = nc.dram_tensor("vkeep", [NC - NX, CG], BF16, kind="ExternalInput")
    wprojT_d = nc.dram_tensor("wprojT", [CG, C], BF16, kind="ExternalInput")
    outT_d = nc.dram_tensor("outT", [C, NX], F32, kind="ExternalOutput")

    with tile.TileContext(nc) as tc:
        with (
            tc.tile_pool(name="persist", bufs=1) as pp,
            tc.tile_pool(name="work", bufs=1) as wp,
            tc.tile_pool(name="wqkc", bufs=8) as wqkp,
            tc.tile_pool(name="attn", bufs=8) as ep,
            tc.tile_pool(name="nrm", bufs=2) as np_pool,
            tc.tile_pool(name="out_sb", bufs=1) as op,
            tc.tile_pool(name="ps", bufs=1, space="PSUM") as psp,
        ):
            # ---- persistent SBUF tiles ----
            q_t = [pp.tile([128, NX], BF16, tag=f"q{i}", name=f"q{i}") for i in range(NP)]
            k_t = [pp.tile([128, NC], BF16, tag=f"k{i}", name=f"k{i}") for i in range(NP)]
            v_t = [pp.tile([128, CG], BF16, tag=f"v{i}", name=f"v{i}") for i in range(NJ)]
            a_t = [pp.tile([128, NX], BF16, tag=f"a{i}", name=f"a{i}") for i in range(NP)]
            bqk_t = pp.tile([128, 8], F32, tag="bqk")
            bv_t = pp.tile([128, CG], F32, tag="bv")
            xT_t = [wp.tile([128, NX], BF16, tag=f"x{i}", name=f"x{i}") for i in range(8)]
            wv_t = [wp.tile([128, CG], BF16, tag=f"wv{i}", name=f"wv{i}") for i in range(8)]
            ones_t = pp.tile([128, 1], BF16, tag="ones")      # denominator lhsT
            onesq_t = pp.tile([128, 128], BF16, tag="onesq")  # init lhsT (all 1)
            erow_t = pp.tile([128, 512], BF16, tag="erow")    # e0 rhs (row0=1)
            zr_t = pp.tile([128, 512], BF16, tag="zr")        # zeros rhs
            proj_sb = [pp.tile([128, NX], F32, tag=f"pj{m}", name=f"pj{m}")
                       for m in range(8)]

            nc.sync.dma_start(bqk_t[:], bqk_d.ap())
            nc.sync.dma_start(bv_t[:], bv_d.ap())

            def body():
                # constants + ACT exp-table preload (hidden under input DMA)
                nc.vector.memset(ones_t[:], 1.0)
                nc.vector.memset(onesq_t[:], 1.0)
                nc.vector.memset(zr_t[:], 0.0)
                nc.vector.memset(erow_t[:], 0.0)
                nc.vector.memset(erow_t[0:1, :], 1.0)
                scr0 = wp.tile([128, 8], F32, tag="scr0")
                scr1 = wp.tile([128, 8], F32, tag="scr1")
                nc.vector.memset(scr0[:], 0.0)
                nc.scalar.activation(scr1[:], scr0[:], EXP)

                # ---- input DMAs, ordered by first use ----
                nc.sync.dma_start(xT_t[0][:], xT_d[0:128, :])
                nc.sync.dma_start(k_t[0][:, NX:NC], kkeepT_d[0:128, :])
                for i in range(1, 8):
                    nc.sync.dma_start(xT_t[i][:], xT_d[i * 128:(i + 1) * 128, :])
                for j in range(NJ // 2, NJ):
                    r0 = (j - NJ // 2) * 128
                    nc.sync.dma_start(v_t[j][:], vkeep_d[r0:r0 + 128, :])
                for i in range(8):
                    nc.sync.dma_start(wv_t[i][:], wvT_d[i * 128:(i + 1) * 128, :])
                for i in range(1, NP):
                    nc.sync.dma_start(k_t[i][:, NX:NC], kkeepT_d[i * 128:(i + 1) * 128, :])

                # ---------- thunk generators ----------
                def qk_thunks(m_list):
                    """QKV q/k m-tiles: m<4 -> q_t[m], m>=4 -> k_t[m-4][:,0:NX].
                    One [128,512] PSUM chunk (bank `qps`) per cch; the m-tile's
                    8 weight K-chunks arrive in ONE strided DMA."""
                    for m in m_list:
                        wqk_m = wqkp.tile([128, 1024], BF16, tag="wqkc",
                                          bufs=3, name=f"wqkm{m}")

                        def wdma(m=m, wqk_m=wqk_m):
                            nc.sync.dma_start(
                                wqk_m[:].rearrange("p (kk c) -> p kk c", kk=8),
                                wqkT_d[0:C, m * 128:(m + 1) * 128]
                                .rearrange("(kk p) c -> p kk c", p=128),
                            )
                        yield wdma
                        for cch in range(2):
                            qps = psp.tile([128, 512], F32, tag="qps", bufs=1,
                                           name=f"qps{m}_{cch}")
                            for kk in range(8):
                                def mm(cch=cch, kk=kk, qps=qps, wqk_m=wqk_m):
                                    nc.tensor.matmul(
                                        qps[:],
                                        wqk_m[:, kk * 128:(kk + 1) * 128],
                                        xT_t[kk][:, cch * 512:(cch + 1) * 512],
                                        start=(kk == 0),
                                        stop=(kk == 7),
                                    )
                                yield mm
                            def bias(m=m, cch=cch, qps=qps):
                                sl = slice(cch * 512, (cch + 1) * 512)
                                dest = q_t[m][:, sl] if m < 4 else k_t[m - 4][:, sl]
                                nc.vector.tensor_scalar_add(dest, qps[:],
                                                            bqk_t[:, m:m + 1])
                            yield bias

                def v_thunks():
                    """V projection m-tiles -> v_t[m] (token-major, channel
                    cols h*64+d)."""
                    for m in range(8):
                        vps = psp.tile([128, 512], F32, tag="qps", bufs=1,
                                       name=f"vps{m}")
                        for kk in range(8):
                            def mm(m=m, kk=kk, vps=vps):
                                nc.tensor.matmul(
                                    vps[:],
                                    xT_t[kk][:, m * 128:(m + 1) * 128],
                                    wv_t[kk][:],
                                    start=(kk == 0),
                                    stop=(kk == 7),
                                )
                            yield mm
                        def fin(m=m, vps=vps):
                            nc.vector.tensor_add(v_t[m][:], vps[:], bv_t[:])
                        yield fin

                def proj_thunks(pairs, first):
                    """Partial output projection over pair K-chunks; partials
                    accumulated in proj_sb (f32 SBUF) via DVE."""
                    for m in range(8):
                        chunks = []
                        for cch in range(2):
                            pps = psp.tile([128, 512], F32, tag="qps", bufs=1,
                                           name=f"pps{m}_{cch}")
                            for i, p in enumerate(pairs):
                                def mm(m=m, cch=cch, p=p, i=i, pps=pps,
                                       chunks=chunks, n=len(pairs)):
                                    if cch == 0:
                                        wpc = op.tile([128, 128], BF16,
                                                      tag="wpc", bufs=8,
                                                      name=f"wpc{p}_{m}")
                                        nc.sync.dma_start(
                                            wpc[:],
                                            wprojT_d[p * 128:(p + 1) * 128,
                                                     m * 128:(m + 1) * 128],
                                        )
                                        chunks.append(wpc)
                                    else:
                                        wpc = chunks[i]
                                    nc.tensor.matmul(
                                        pps[:],
                                        wpc[:],
                                        a_t[p][:, cch * 512:(cch + 1) * 512],
                                        start=(i == 0),
                                        stop=(i == n - 1),
                                    )
                                yield mm
                            def fin(m=m, cch=cch, pps=pps, first=first):
                                sl = slice(cch * 512, (cch + 1) * 512)
                                if first:
                                    nc.vector.tensor_copy(proj_sb[m][:, sl],
                                                          pps[:])
                                else:
                                    nc.vector.tensor_add(proj_sb[m][:, sl],
                                                         proj_sb[m][:, sl],
                                                         pps[:])
                            yield fin

                # Drained thunks provide NO automatic ordering: Tile deps
                # follow program order, so a consumer emitted before its
                # producer thunk reads stale/uninitialized data.  Thunks are
                # tagged with a completion marker; need(marker) force-drains
                # (in FIFO order, keeping PSUM chains contiguous) until that
                # marker's thunks have all been emitted.
                pending = []          # list of (marker_or_None, thunk)
                done = set()

                def _pop():
                    marker, th = pending.pop(0)
                    th()
                    if marker:
                        done.add(marker)

                def drain(n):
                    for _ in range(n):
                        if pending:
                            _pop()

                def need(marker):
                    while marker not in done and pending:
                        _pop()

                def tag_last(marker, thunks):
                    thunks = list(thunks)
                    return [(None, th) for th in thunks[:-1]] + \
                           [(marker, thunks[-1])]

                # ---------- prologue: q/k for pair 0 ----------
                for th in qk_thunks([0, 4]):
                    th()

                pending += tag_last("qk1", qk_thunks([1, 5]))
                vt = list(v_thunks())
                for m in range(8):
                    pending += tag_last(f"v{m}", vt[m * 9:(m + 1) * 9])
                pending += tag_last("qk2", qk_thunks([2, 6]))
                pending += tag_last("qk3", qk_thunks([3, 7]))

                # ---------- attention over head pairs ----------
                for hp in range(NP):
                    if hp >= 1:
                        need(f"qk{hp}")        # q_t/k_t[hp] producers
                    av_lag = 4 if hp == 0 else 1
                    d1, d2 = (3, 3) if hp == 0 else (1, 2)
                    jo = (list(range(NJ // 2, NJ)) + list(range(NJ // 2))
                          if hp == 0 else list(range(NJ)))
                    av = psp.tile([128, NX], F32, tag="av", bufs=1,
                                  name=f"av{hp}")
                    den = psp.tile([128, 512], F32, tag="den", bufs=1,
                                   name=f"den{hp}")
                    # bank init matmuls: av <- ones.T@0 = 0, den <- ones.T@e0 = 1
                    for cch in range(2):
                        nc.tensor.matmul(
                            av[:, cch * 512:(cch + 1) * 512],
                            onesq_t[:], zr_t[:], start=True, stop=True,
                        )
                    nc.tensor.matmul(den[:], onesq_t[:], erow_t[:],
                                     start=True, stop=True)

                    ets = {}

                    def av_den(j, last, hp=hp, av=av, den=den, ets=ets):
                        """AV pair + 4-way col-packed denominator matmuls,
                        interleaved so consecutive matmuls hit disjoint PE
                        col-groups (AV half 0 <-> den of half 1, etc.) and can
                        overlap on the array."""
                        if j < NJ // 2:
                            need(f"v{j}")      # v_t[j] producer must precede
                        etA, etB = ets.pop(j)

                        def av_mm(half, cch, et):
                            h = 2 * hp + half
                            sl = slice(cch * 512, (cch + 1) * 512)
                            nc.tensor.matmul(
                                av[half * 64:(half + 1) * 64, sl],
                                v_t[j][:, h * 64:(h + 1) * 64],
                                et[:, sl],
                                start=False, stop=(last and cch == 1),
                                skip_group_check=True,
                            )

                        def den_mm(half, cch, et):
                            idx = half * 2 + cch
                            nc.tensor.matmul(
                                den[idx * 32:idx * 32 + 1, :],
                                ones_t[:],
                                et[:, cch * 512:(cch + 1) * 512],
                                start=False, stop=last,
                                skip_group_check=True,
                                tile_position=(0, idx * 32),
                            )

                        av_mm(0, 0, etA)
                        den_mm(1, 0, etB)
                        av_mm(1, 0, etB)
                        den_mm(0, 0, etA)
                        av_mm(0, 1, etA)
                        den_mm(1, 1, etB)
                        av_mm(1, 1, etB)
                        den_mm(0, 1, etA)

                    for step, j in enumerate(jo):
                        spsA = psp.tile([128, NX], F32, tag="spsA", bufs=1,
                                        name=f"spsA{hp}_{j}")
                        spsB = psp.tile([128, NX], F32, tag="spsB", bufs=1,
                                        name=f"spsB{hp}_{j}")
                        etA = ep.tile([128, NX], BF16, tag="etA",
                                      name=f"etA{hp}_{j}")
                        etB = ep.tile([128, NX], BF16, tag="etB",
                                      name=f"etB{hp}_{j}")
                        ets[j] = (etA, etB)
                        for cch in range(2):
                            nc.tensor.matmul(
                                spsA[:, cch * 512:(cch + 1) * 512],
                                k_t[hp][0:64, j * 128:(j + 1) * 128],
                                q_t[hp][0:64, cch * 512:(cch + 1) * 512],
                                start=True, stop=True,
                            )
                        nc.scalar.activation(etA[:], spsA[:], EXP, scale=SCALE)
                        if step >= av_lag:
                            av_den(jo[step - av_lag], last=False)
                        drain(d1)
                        for cch in range(2):
                            nc.tensor.matmul(
                                spsB[:, cch * 512:(cch + 1) * 512],
                                k_t[hp][64:128, j * 128:(j + 1) * 128],
                                q_t[hp][64:128, cch * 512:(cch + 1) * 512],
                                start=True, stop=True,
                            )
                        nc.scalar.activation(etB[:], spsB[:], EXP, scale=SCALE)
                        drain(d2)
                    for step in range(NJ - av_lag, NJ):
                        av_den(jo[step], last=(step == NJ - 1))
                        drain(1)

                    # ---- normalization for this pair ----
                    avs = np_pool.tile([128, NX], F32, tag="avs", bufs=2,
                                       name=f"avs{hp}")
                    nc.vector.tensor_copy(avs[:], av[:])
                    dens = np_pool.tile([128, 512], F32, tag="dens", bufs=2,
                                        name=f"dens{hp}")
                    # dens = max(den - 1, tiny): removes the bank-init +1
                    # exactly; `tiny` keeps never-written rows finite under
                    # reciprocal.
                    nc.vector.tensor_scalar(
                        dens[:], den[:], 1.0, 1e-30,
                        op0=mybir.AluOpType.subtract,
                        op1=mybir.AluOpType.max,
                    )
                    # gpsimd partition_broadcast only honors sources at
                    # partition 0 on HW, so first move each denominator row
                    # into segments of a partition-0 tile via 1-partition DVE
                    # reciprocals (DVE handles mismatched base partitions when
                    # the partition counts match).
                    d4 = np_pool.tile([1, 4 * 512], F32, tag="d4", bufs=2,
                                      name=f"d4{hp}")
                    for r in range(4):
                        nc.vector.reciprocal(
                            d4[0:1, r * 512:(r + 1) * 512],
                            dens[r * 32:r * 32 + 1, :],
                        )
                    rb = np_pool.tile([128, NX], F32, tag="rb", bufs=2,
                                      name=f"rb{hp}")
                    for half in range(2):
                        for cch in range(2):
                            r = half * 2 + cch
                            nc.gpsimd.partition_broadcast(
                                rb[half * 64:(half + 1) * 64,
                                   cch * 512:(cch + 1) * 512],
                                d4[0:1, r * 512:(r + 1) * 512],
                            )
                    nc.vector.tensor_mul(a_t[hp][:], avs[:], rb[:])

                    if hp == 2:
                        pending += [(None, th)
                                    for th in proj_thunks([0, 1, 2], first=True)]

                while pending:
                    _pop()

                # ---------- tail: last proj K-chunk + store ----------
                for m in range(8):
                    wpc = op.tile([128, 128], BF16, tag="wpc", bufs=8,
                                  name=f"wpcT{m}")
                    nc.sync.dma_start(
                        wpc[:],
                        wprojT_d[3 * 128:4 * 128, m * 128:(m + 1) * 128],
                    )
                    for cch in range(2):
                        pps = psp.tile([128, 512], F32, tag="qps", bufs=1,
                                       name=f"ppsT{m}_{cch}")
                        nc.tensor.matmul(
                            pps[:], wpc[:],
                            a_t[3][:, cch * 512:(cch + 1) * 512],
                            start=True, stop=True,
                        )
                        ot = op.tile([128, 512], F32, tag="ot", bufs=4)
                        nc.vector.tensor_add(
                            ot[:], pps[:],
                            proj_sb[m][:, cch * 512:(cch + 1) * 512])
                        nc.sync.dma_start(
                            outT_d[m * 128:(m + 1) * 128,
                                   cch * 512:(cch + 1) * 512],
                            ot[:],
                        )

            if reps == 1:
                body()
            else:
                hints = (
                    mybir.EngineType.PE,
                    mybir.EngineType.Activation,
                    mybir.EngineType.DVE,
                    mybir.EngineType.SP,
                )
                with tc.For_i(0, reps, 1, hint_engines=hints):
                    body()

    nc.compile()
    return nc


def _get_nc():
    if "nc" not in _STATE:
        _STATE["nc"] = _build()
    return _STATE["nc"]


def _prep_in_maps(x, update_idx, cache_k, cache_v, w_qkv, b_qkv):
    """Host-side sharding: build the 8 per-core input dicts."""
    import ml_dtypes

    x = np.asarray(x, np.float32)
    update_idx = np.asarray(update_idx)
    cache_k = np.asarray(cache_k, np.float32)
    cache_v = np.asarray(cache_v, np.float32)
    w_qkv = np.asarray(w_qkv, np.float32)
    b_qkv = np.asarray(b_qkv, np.float32)

    per_g = []
    for g in range(G):
        qs = slice(g * CG, (g + 1) * CG)
        ks = slice(C + g * CG, C + (g + 1) * CG)
        vs = slice(2 * C + g * CG, 2 * C + (g + 1) * CG)
        wqkT = np.ascontiguousarray(
            np.concatenate([w_qkv[qs], w_qkv[ks]], 0).T
        ).astype(ml_dtypes.bfloat16)                                 # (C, 2CG)
        wvT = np.ascontiguousarray(w_qkv[vs].T).astype(ml_dtypes.bfloat16)
        bqk = np.ascontiguousarray(
            np.concatenate([b_qkv[qs], b_qkv[ks]]).reshape(8, 128).T
        )                                                            # (128, 8)
        bv = np.broadcast_to(b_qkv[vs][None, :], (128, CG)).copy()
        per_g.append((wqkT, wvT, bqk, bv))

    in_maps = []
    for b in range(B):
        idx = update_idx[b]
        mask = np.ones(NC, bool)
        mask[idx] = False
        keep = np.nonzero(mask)[0]                                   # sorted
        xT = np.ascontiguousarray(x[b].T).astype(ml_dtypes.bfloat16)  # (C, NX)
        for g in range(G):
            wqkT, wvT, bqk, bv = per_g[g]
            hsel = slice(g * HPG, (g + 1) * HPG)
            kk = cache_k[b, hsel][:, keep, :]                        # (HPG, NC-NX, DH)
            kkeepT = np.ascontiguousarray(
                kk.transpose(0, 2, 1).reshape(HPG * DH, NC - NX)
            ).astype(ml_dtypes.bfloat16)
            vkeep = np.ascontiguousarray(
                cache_v[b, hsel][:, keep, :].transpose(1, 0, 2)
                .reshape(NC - NX, CG)
            ).astype(ml_dtypes.bfloat16)
            wprojT = np.asarray(_STATE["wprojT"][g], ml_dtypes.bfloat16)
            in_maps.append(
                dict(
                    xT=xT, wqkT=wqkT, wvT=wvT, bqk=bqk, bv=bv,
                    kkeepT=kkeepT, vkeep=vkeep, wprojT=wprojT,
                )
            )
    return in_maps


def kernel(x, update_idx, cache_k, cache_v, w_qkv, b_qkv, w_proj, b_proj):
    nc = _get_nc()
    w_proj = np.asarray(w_proj, np.float32)
    b_proj = np.asarray(b_proj, np.float32)
    _STATE["wprojT"] = [
        np.ascontiguousarray(w_proj[:, g * CG:(g + 1) * CG].T) for g in range(G)
    ]
    in_maps = _prep_in_maps(x, update_idx, cache_k, cache_v, w_qkv, b_qkv)
    res = bass_utils.run_bass_kernel_spmd(nc, in_maps, core_ids=list(range(N_CORES)))
    _STATE["last_results"] = res
    out = np.empty((B, NX, C), np.float32)
    for b in range(B):
        acc = res.results[2 * b]["outT"] + res.results[2 * b + 1]["outT"]
        out[b] = acc.T + b_proj
    return out
